# revision 24
# baseline (speedup 1.0000x reference)
import os
import sys

for _p in ("/opt/trn_rl_repo", "/root/.axon_site/_ro/trn_rl_repo"):
    if _p not in sys.path:
        sys.path.insert(0, _p)

import numpy as np

import concourse.bass as bass
import concourse.tile as tile
from concourse import bacc, mybir
from concourse.bass_utils import run_bass_kernel_spmd

try:
    import ml_dtypes

    BF16 = ml_dtypes.bfloat16
except Exception:  # pragma: no cover
    BF16 = np.float32

HEADS = 8
N_CORES = 8
B, C, T, H, W = 2, 192, 8, 64, 64
RH = H // N_CORES           # 8 rows per slice/chunk
NJ = 32                     # jobs per core: 16 inter + 16 intra
FR, WP = RH + 2, W + 2      # 10 x 66 padded job tile
FSZ = FR * WP               # 660
OSZ = RH * W                # 512
CC3 = 3 * C                 # 576
CBS = [128, 128, 128, 128, 64]
QK_CBS = (0, 1, 2)
V_CBS = (3, 4)
NTAP_I = 28                 # 27 taps + bias
NTAP_T = 10                 # 9 taps + bias
PAYC = 194

F32 = mybir.dt.float32
BF = mybir.dt.bfloat16

_cached = {}
KS = int(os.environ.get("KS", "5"))
KPAY = int(os.environ.get("KPAY", "1"))   # 0: skip pay TTR evict
KGRAM = int(os.environ.get("KGRAM", "1"))  # 0: skip gram matmuls
KTRN = int(os.environ.get("KTRN", "1"))   # 0: skip dma transposes
# inter-branch DW channel blocks offloaded to DVE (STT chains)
KDVE = tuple(int(c) for c in os.environ.get("KDVE", "1,2").split(",") if c)


def _taps_inter():
    return [(dt, dh, dw) for dt in (-1, 0, 1) for dh in (-1, 0, 1)
            for dw in (-1, 0, 1)]


def _taps_intra():
    return [(dh, dw) for dh in (-1, 0, 1) for dw in (-1, 0, 1)]


def _build_program():
    key = ("nc", KS)
    if key in _cached:
        return _cached[key]

    nc = bacc.Bacc("TRN2", target_bir_lowering=False, debug=False,
                   num_devices=N_CORES)

    def din(name, shape, dt):
        return nc.dram_tensor(name, shape, dt, kind="ExternalInput").ap()

    xa_d = din("xa", [128, NJ, FSZ], BF)
    xb_d = din("xb", [65, NJ, FSZ], BF)       # row 64 = validity mask
    wpw = {"i": (din("wpw_i_a", [128, CC3], BF), din("wpw_i_b", [65, CC3], BF)),
           "t": (din("wpw_t_a", [128, CC3], BF), din("wpw_t_b", [65, CC3], BF))}
    diag_d = {"i": din("diag_i", [128, 5, NTAP_I, 128], BF),
              "t": din("diag_t", [128, 5, NTAP_T, 128], BF)}
    wsc_d = din("wsc_i", [128, 5, NTAP_I], F32)
    wpT = {"i": (din("wpT_i_a", [128, 192], BF), din("wpT_i_b", [64, 192], BF)),
           "t": (din("wpT_t_a", [128, 192], BF), din("wpT_t_b", [64, 192], BF))}
    temp2_d = din("temp2", [128, 2], F32)
    maskf_d = din("mask_f", [128, 2, 192], F32)
    identf_d = din("ident_f", [128, 2, 192], F32)
    biaso_d = din("bias_out", [128, 4], F32)

    out_i = nc.dram_tensor("out_i", [192, 16, OSZ], F32,
                           kind="ExternalOutput").ap()
    out_t = nc.dram_tensor("out_t", [192, 16, OSZ], F32,
                           kind="ExternalOutput").ap()
    cc_in = nc.dram_tensor("cc_in", [B, 192, PAYC], F32, kind="Internal").ap()
    cc_out = nc.dram_tensor("cc_out", [B, 192, PAYC], F32, kind="Internal",
                            addr_space="Shared").ap()
    rk_scr = nc.dram_tensor("rk_scr", [4, 192], F32, kind="Internal").ap()

    with tile.TileContext(nc) as tc:
        import contextlib

        ctx = contextlib.ExitStack()
        with ctx:
            const = ctx.enter_context(tc.tile_pool(name="const", bufs=1))
            xsp = ctx.enter_context(tc.tile_pool(name="xsp", bufs=3))
            pwp = ctx.enter_context(tc.tile_pool(name="pwp", bufs=4))
            dqp = ctx.enter_context(tc.tile_pool(name="dqp", bufs=3))
            qtp = ctx.enter_context(tc.tile_pool(name="qtp", bufs=2))
            vip = ctx.enter_context(tc.tile_pool(name="vip", bufs=1))
            vtp = ctx.enter_context(tc.tile_pool(name="vtp", bufs=2))
            payp = ctx.enter_context(tc.tile_pool(name="payp", bufs=1))
            smx = ctx.enter_context(tc.tile_pool(name="smx", bufs=2))
            mtp = ctx.enter_context(tc.tile_pool(name="mtp", bufs=2))
            outp = ctx.enter_context(tc.tile_pool(name="outp", bufs=2))
            accp = ctx.enter_context(tc.tile_pool(name="accp", bufs=2))

            ps_pw = ctx.enter_context(
                tc.tile_pool(name="ps_pw", bufs=2, space="PSUM"))
            ps_dw = ctx.enter_context(
                tc.tile_pool(name="ps_dw", bufs=2, space="PSUM"))
            ps_g = ctx.enter_context(
                tc.tile_pool(name="ps_g", bufs=1, space="PSUM"))

            def load(ap_in, dt, tag):
                t_ = const.tile(list(ap_in.shape), dt, tag=tag)
                nc.sync.dma_start(t_[:], ap_in)
                return t_

            wpw_s = {k: (load(v[0], BF, f"wpwa{k}"), load(v[1], BF, f"wpwb{k}"))
                     for k, v in wpw.items()}
            diag_s = {k: load(v, BF, f"diag{k}") for k, v in diag_d.items()}
            wsc_s = load(wsc_d, F32, "wsc")
            wpT_s = {k: (load(v[0], BF, f"wpta{k}"), load(v[1], BF, f"wptb{k}"))
                     for k, v in wpT.items()}
            temp2_s = load(temp2_d, F32, "temp2")
            maskf_s = load(maskf_d, F32, "maskf")
            identf_s = load(identf_d, F32, "identf")
            biaso_s = load(biaso_d, F32, "biaso")

            ones512 = const.tile([128, OSZ], BF, tag="ones512")
            nc.vector.memset(ones512[:], 1.0)
            zf = const.tile([128, FR, WP], BF, tag="zf")
            nc.vector.memset(zf[:], 0.0)

            pay_a = payp.tile([128, 4, PAYC], F32, tag="pay_a")
            pay_b = payp.tile([64, 4, PAYC], F32, tag="pay_b")

            v_i = (vip.tile([128, 16, OSZ], BF, tag="via", name="via"),
                   vip.tile([64, 16, OSZ], BF, tag="vib", name="vib"))

            mt_i = [(mtp.tile([128, 192], BF, tag="mtia", name=f"mtia{b}"),
                     mtp.tile([64, 192], BF, tag="mtib", name=f"mtib{b}"))
                    for b in range(B)]

            evict_ctr = [0]

            def evict(dst_ap, src_ap):
                evict_ctr[0] += 1
                if evict_ctr[0] % 2 == 0:
                    nc.vector.tensor_copy(dst_ap, src_ap)
                else:
                    nc.scalar.copy(dst_ap, src_ap)

            pw_ring = {}

            def do_pw(job):
                br = "i" if job < 16 else "t"
                wa, wb = wpw_s[br]
                xt_a = xsp.tile([128, FSZ], BF, tag="xa")
                xt_b = xsp.tile([65, FSZ], BF, tag="xb")
                nc.sync.dma_start(xt_a[:], xa_d[:, job, :])
                nc.sync.dma_start(xt_b[:], xb_d[:, job, :])
                pwt = pwp.tile([128, 5, FR, WP], BF, tag="pw")
                pw_ring[job] = pwt
                for cb in range(5):
                    psz = CBS[cb]
                    c0 = cb * 128
                    for half in range(2):
                        s0, s1 = half * 330, half * 330 + 330
                        ps = ps_pw.tile([psz, 330], F32, tag="pw")
                        nc.tensor.matmul(ps[:], wa[:, c0:c0 + psz],
                                         xt_a[:, s0:s1], start=True, stop=False)
                        nc.tensor.matmul(ps[:], wb[:, c0:c0 + psz],
                                         xt_b[:, s0:s1], start=False, stop=True)
                        dst = pwt[:psz, cb].rearrange("p a b -> p (a b)")
                        evict(dst[:, s0:s1], ps[:])

            def dw_moving(job, cb, tap):
                # shifted view of pw output for one tap (or None -> zeros)
                psz = CBS[cb]
                br_inter = job < 16
                if br_inter:
                    dt_, dh, dw = tap
                    b, t = job // 8, job % 8
                    tt = t + dt_
                    if 0 <= tt < T:
                        src = pw_ring[b * 8 + tt]
                        return src[:psz, cb, 1 + dh:9 + dh, 1 + dw:65 + dw]
                    return zf[:psz, 1 + dh:9 + dh, 1 + dw:65 + dw]
                dh, dw = tap
                return pw_ring[job][:psz, cb, 1 + dh:9 + dh, 1 + dw:65 + dw]

            def do_dw_gram(job, v_cur):
                br = "i" if job < 16 else "t"
                taps = _taps_inter() if br == "i" else _taps_intra()
                ntap = len(taps) + 1
                dg = diag_s[br]
                inst = job // 8
                first_j = job % 8 == 0
                last_j = job % 8 == 7
                qkT = None
                for cb in range(5):
                    psz = CBS[cb]
                    c0 = cb * 128
                    use_dve = (br == "i" and cb in KDVE and cb in QK_CBS
                               and KS >= 2)
                    if use_dve:
                        # STT chain on DVE: acc = sum_t w_t * shift_t(pw)
                        acc = [accp.tile([128, OSZ], BF, tag="acc0",
                                         name="acc0"),
                               accp.tile([128, OSZ], BF, tag="acc1",
                                         name="acc1")]
                        dq = dqp.tile([128, OSZ], BF, tag="dq")
                        nc.vector.tensor_scalar(
                            acc[0][:psz], dw_moving(job, cb, taps[0]),
                            wsc_s[:psz, cb, 0:1],
                            wsc_s[:psz, cb, ntap - 1:ntap],
                            op0=mybir.AluOpType.mult,
                            op1=mybir.AluOpType.add)
                        cur = 0
                        for ti in range(1, len(taps)):
                            last = ti == len(taps) - 1
                            out_ap = dq[:psz] if last else acc[1 - cur][:psz]
                            nc.vector.scalar_tensor_tensor(
                                out_ap, dw_moving(job, cb, taps[ti]),
                                wsc_s[:psz, cb, ti:ti + 1], acc[cur][:psz],
                                op0=mybir.AluOpType.mult,
                                op1=mybir.AluOpType.add)
                            cur = 1 - cur
                        if KS < 3:
                            continue
                        if qkT is None:
                            qkT = qtp.tile([128, 4, 384], BF, tag="qkT")
                            if KTRN == 0:
                                nc.vector.memset(qkT[:], 0.0)
                        if KTRN:
                            for ch in range(4):
                                nc.sync.dma_start_transpose(
                                    qkT[:, ch, c0:c0 + psz],
                                    dq[:psz, ch * 128:(ch + 1) * 128])
                        continue
                    dps = ps_dw.tile([psz, OSZ], F32, tag="dw")
                    for ti in range(ntap):
                        if ti < len(taps):
                            r_ap = dw_moving(job, cb, taps[ti])
                        else:
                            r_ap = ones512[:psz, :]
                        nc.tensor.matmul(dps[:], dg[:psz, cb, ti, :psz], r_ap,
                                         start=(ti == 0), stop=(ti == ntap - 1))
                    if cb in QK_CBS:
                        if KS < 2:
                            continue
                        dq = dqp.tile([128, OSZ], BF, tag="dq")
                        evict(dq[:psz], dps[:])
                        if KS < 3:
                            continue
                        if qkT is None:
                            qkT = qtp.tile([128, 4, 384], BF, tag="qkT")
                            if KTRN == 0:
                                nc.vector.memset(qkT[:], 0.0)
                        if KTRN:
                            for ch in range(4):
                                nc.sync.dma_start_transpose(
                                    qkT[:, ch, c0:c0 + psz],
                                    dq[:psz, ch * 128:(ch + 1) * 128])
                    else:
                        if KS < 2:
                            continue
                        va, vb = v_cur
                        slot = job if job < 16 else (job - 16) % 8
                        dst = va if cb == 3 else vb
                        evict(dst[:psz, slot, :], dps[:])
                # gram accumulation for this job
                if KS < 3 or KGRAM == 0:
                    return
                g_qa, g_qb, g_ka, g_kb = gram_tiles[inst]
                for ch in range(4):
                    qt = qkT[:, ch, :]
                    st = first_j and ch == 0
                    sp = last_j and ch == 3
                    # [q.q | q.k] fused: cols 0:192 = q gram, 192:384 = qk
                    nc.tensor.matmul(g_qa[:], qt[:, 0:128],
                                     qt[:, 0:384], start=st, stop=sp)
                    nc.tensor.matmul(g_qb[:], qt[:, 128:192],
                                     qt[:, 0:384], start=st, stop=sp)
                    nc.tensor.matmul(g_ka[:], qt[:, 192:320],
                                     qt[:, 192:384], start=st, stop=sp)
                    nc.tensor.matmul(g_kb[:], qt[:, 320:384],
                                     qt[:, 192:384], start=st, stop=sp)

            gram_tiles = {}

            def alloc_gram(inst):
                gram_tiles[inst] = (
                    ps_g.tile([128, 384], F32, tag="gqa", name=f"gqa{inst}"),
                    ps_g.tile([64, 384], F32, tag="gqb", name=f"gqb{inst}"),
                    ps_g.tile([128, 192], F32, tag="gka", name=f"gka{inst}"),
                    ps_g.tile([64, 192], F32, tag="gkb", name=f"gkb{inst}"),
                )

            def pay_evict(inst):
                if KPAY == 0 or KGRAM == 0:
                    return
                g_qa, g_qb, g_ka, g_kb = gram_tiles[inst]
                nc.vector.tensor_copy(pay_a[:, inst, 0:192], g_qa[:, 192:384])
                nc.scalar.copy(pay_b[:, inst, 0:192], g_qb[:, 192:384])
                tmp = smx.tile([128, 192], F32, tag="diag_tmp")
                for (g, ident, dst) in (
                        (g_qa[:, 0:192], identf_s[:, 0, :],
                         pay_a[:, inst, 192:193]),
                        (g_qb[:, 0:192], identf_s[0:64, 1, :],
                         pay_b[:, inst, 192:193]),
                        (g_ka[:], identf_s[:, 0, :],
                         pay_a[:, inst, 193:194]),
                        (g_kb[:], identf_s[0:64, 1, :],
                         pay_b[:, inst, 193:194])):
                    p = g.shape[0]
                    nc.vector.tensor_mul(tmp[:p], g, ident)
                    nc.vector.tensor_reduce(dst, tmp[:p],
                                            axis=mybir.AxisListType.X,
                                            op=mybir.AluOpType.add)

            def posts(inst, br, pa, pb, mt_a, mt_b):
                # pa [128, PAYC] f32 AP; pb [64, PAYC]
                rq_a = smx.tile([128, 1], F32, tag="rq_a")
                rq_b = smx.tile([64, 1], F32, tag="rq_b")
                rk_a = smx.tile([128, 1], F32, tag="rk_a")
                rk_b = smx.tile([64, 1], F32, tag="rk_b")
                for (dst, src, pt, p) in ((rq_a, pa, 0, 128), (rq_b, pb, 1, 64)):
                    nc.scalar.sqrt(dst[:], src[:, 192:193])
                    nc.vector.reciprocal(dst[:], dst[:])
                    nc.vector.tensor_scalar_mul(dst[:], dst[:],
                                                temp2_s[:p, pt:pt + 1])
                for (dst, src) in ((rk_a, pa), (rk_b, pb)):
                    nc.scalar.sqrt(dst[:], src[:, 193:194])
                    nc.vector.reciprocal(dst[:], dst[:])
                nc.sync.dma_start(rk_scr[inst, 0:128], rk_a[:])
                nc.sync.dma_start(rk_scr[inst, 128:192], rk_b[:])
                rkb_a = smx.tile([128, 192], F32, tag="rkb_a")
                rkb_b = smx.tile([64, 192], F32, tag="rkb_b")
                nc.sync.dma_start(
                    rkb_a[:], bass.AP(tensor=rk_scr.tensor,
                                      offset=rk_scr.offset + inst * 192,
                                      ap=[[0, 128], [1, 192]]))
                nc.sync.dma_start(
                    rkb_b[:], bass.AP(tensor=rk_scr.tensor,
                                      offset=rk_scr.offset + inst * 192,
                                      ap=[[0, 64], [1, 192]]))
                attn_a = smx.tile([128, 192], BF, tag="attn_a")
                attn_b = smx.tile([64, 192], BF, tag="attn_b")
                for (pt, p, post, rqv, rkb, attn) in (
                        (0, 128, pa, rq_a, rkb_a, attn_a),
                        (1, 64, pb, rq_b, rkb_b, attn_b)):
                    s = smx.tile([128, 192], F32, tag="s_f32")
                    nc.vector.tensor_scalar_mul(s[:p], post[:, 0:192], rqv[:p])
                    nc.vector.tensor_mul(s[:p], s[:p], rkb[:p])
                    nc.scalar.activation(s[:p], s[:p],
                                         mybir.ActivationFunctionType.Exp)
                    rs = smx.tile([128, 1], F32, tag="rs")
                    e = smx.tile([128, 192], F32, tag="e_f32")
                    nc.vector.tensor_mul(e[:p], s[:p], maskf_s[:p, pt, :])
                    nc.vector.tensor_reduce(rs[:p], e[:p],
                                            axis=mybir.AxisListType.X,
                                            op=mybir.AluOpType.add)
                    nc.vector.reciprocal(rs[:p], rs[:p])
                    nc.vector.tensor_scalar_mul(attn[:], e[:p], rs[:p])
                wpa, wpb = wpT_s[br]
                mps_a = ps_g.tile([128, 192], F32, tag="gka")
                mps_b = ps_g.tile([64, 192], F32, tag="gkb")
                nc.tensor.matmul(mps_a[:], attn_a[:, 0:128], wpa[:],
                                 start=True, stop=False)
                nc.tensor.matmul(mps_a[:], attn_b[:, 0:128], wpb[:],
                                 start=False, stop=True)
                nc.tensor.matmul(mps_b[:], attn_a[:, 128:192], wpa[:],
                                 start=True, stop=False)
                nc.tensor.matmul(mps_b[:], attn_b[:, 128:192], wpb[:],
                                 start=False, stop=True)
                evict(mt_a[:], mps_a[:])
                evict(mt_b[:], mps_b[:])

            def attnv(job_slot, mt_a, mt_b, va, vb, out_d, out_slot, bc):
                eo_a = ps_dw.tile([128, OSZ], F32, tag="dw")
                eo_b = ps_dw.tile([64, OSZ], F32, tag="dw")
                nc.tensor.matmul(eo_a[:], mt_a[:, 0:128], va[:, job_slot, :],
                                 start=True, stop=False)
                nc.tensor.matmul(eo_a[:], mt_b[:, 0:128], vb[:, job_slot, :],
                                 start=False, stop=True)
                nc.tensor.matmul(eo_b[:], mt_a[:, 128:192], va[:, job_slot, :],
                                 start=True, stop=False)
                nc.tensor.matmul(eo_b[:], mt_b[:, 128:192], vb[:, job_slot, :],
                                 start=False, stop=True)
                oa = outp.tile([128, OSZ], F32, tag="oa")
                ob = outp.tile([64, OSZ], F32, tag="ob")
                nc.scalar.activation(oa[:], eo_a[:],
                                     mybir.ActivationFunctionType.Identity,
                                     bias=biaso_s[:, bc:bc + 1], scale=1.0)
                nc.scalar.activation(ob[:], eo_b[:],
                                     mybir.ActivationFunctionType.Identity,
                                     bias=biaso_s[0:64, bc + 1:bc + 2],
                                     scale=1.0)
                nc.sync.dma_start(out_d[0:128, out_slot, :], oa[:])
                nc.sync.dma_start(out_d[128:192, out_slot, :], ob[:])

            # ================= phase A: inter =========================
            alloc_gram(0)
            for job in range(16):
                if job == 8:
                    alloc_gram(1)
                do_pw(job)
                if job >= 1:
                    do_dw_gram(job - 1, v_i)
                if job == 8 and KS >= 3:
                    pay_evict(0)
            do_dw_gram(15, v_i)
            if KS >= 3:
                pay_evict(1)
            if KS >= 4:
                nc.sync.dma_start(
                    cc_in[:, 0:128, :].rearrange("i p c -> p i c"),
                    pay_a[:, 0:B, :])
                nc.sync.dma_start(
                    cc_in[:, 128:192, :].rearrange("i p c -> p i c"),
                    pay_b[:, 0:B, :])
                nc.gpsimd.collective_compute(
                    "AllReduce", mybir.AluOpType.add,
                    replica_groups=[list(range(N_CORES))],
                    ins=[cc_in[:]], outs=[cc_out[:]])

            def phase_c():
                # inter posts + attn@v; issued mid-phase-B so it overlaps
                for b in range(B):
                    pa = smx.tile([128, PAYC], F32, tag="cc_a")
                    pb = smx.tile([64, PAYC], F32, tag="cc_b")
                    nc.sync.dma_start(pa[:], cc_out[b, 0:128, :])
                    nc.sync.dma_start(pb[:], cc_out[b, 128:192, :])
                    posts(b, "i", pa[:], pb[:], mt_i[b][0], mt_i[b][1])
                for job in range(16):
                    b = job // 8
                    attnv(job, mt_i[b][0], mt_i[b][1], v_i[0], v_i[1],
                          out_i, job, 0)

            # ================= phase B: intra =========================
            for F in range(2):
                inst = 2 + F
                alloc_gram(inst)
                v_cur = (vtp.tile([128, 8, OSZ], BF, tag="vta",
                                  name=f"vta{F}"),
                         vtp.tile([64, 8, OSZ], BF, tag="vtb",
                                  name=f"vtb{F}"))
                for jj in range(8):
                    job = 16 + 8 * F + jj
                    do_pw(job)
                    do_dw_gram(job, v_cur)
                if KS >= 3:
                    pay_evict(inst)
                if F == 0 and KS >= 5:
                    phase_c()
                if KS >= 5:
                    mt_t_a = mtp.tile([128, 192], BF, tag="mtta")
                    mt_t_b = mtp.tile([64, 192], BF, tag="mttb")
                    posts(inst, "t", pay_a[:, inst, :], pay_b[:, inst, :],
                          mt_t_a, mt_t_b)
                    for jj in range(8):
                        attnv(jj, mt_t_a, mt_t_b, v_cur[0], v_cur[1],
                              out_t, 8 * F + jj, 2)

    nc.compile()
    _cached[key] = nc
    return nc


# --------------------------------------------------------------------------
# host-side input prep
# --------------------------------------------------------------------------

def _prep_core_inputs(inputs):
    x = np.asarray(inputs["x"], np.float32)           # [B, C, T, H, W]
    temp = np.asarray(inputs["temperature"], np.float32).reshape(HEADS)

    def pw_mats(w, bvec):
        wt = np.asarray(w, np.float32).reshape(CC3, C).T       # [ic, oc]
        wa = wt[0:128]
        wb = np.zeros((65, CC3), np.float32)
        wb[0:64] = wt[128:192]
        wb[64] = np.asarray(bvec, np.float32)
        return wa.astype(BF16), wb.astype(BF16)

    wpw_i_a, wpw_i_b = pw_mats(inputs["qkv_inter_w"], inputs["qkv_inter_b"])
    wpw_t_a, wpw_t_b = pw_mats(inputs["qkv_intra_w"], inputs["qkv_intra_b"])

    def diag_mat(w, bvec, ntap):
        w = np.asarray(w, np.float32).reshape(CC3, -1)         # [576, taps-1]
        bvec = np.asarray(bvec, np.float32)
        arr = np.zeros((128, 5, ntap, 128), np.float32)
        for cb in range(5):
            psz = CBS[cb]
            for p in range(psz):
                ch = cb * 128 + p
                arr[p, cb, :ntap - 1, p] = w[ch]
                arr[p, cb, ntap - 1, p] = bvec[ch]
        return arr.astype(BF16)

    diag_i = diag_mat(inputs["qkv_inter_dw_w"], inputs["qkv_inter_dw_b"],
                      NTAP_I)
    diag_t = diag_mat(inputs["qkv_intra_dw_w"], inputs["qkv_intra_dw_b"],
                      NTAP_T)

    def wsc_mat(w, bvec, ntap):
        w = np.asarray(w, np.float32).reshape(CC3, -1)
        bvec = np.asarray(bvec, np.float32)
        arr = np.zeros((128, 5, ntap), np.float32)
        for cb in range(5):
            psz = CBS[cb]
            arr[:psz, cb, :ntap - 1] = w[cb * 128:cb * 128 + psz]
            arr[:psz, cb, ntap - 1] = bvec[cb * 128:cb * 128 + psz]
        return arr

    wsc_i = wsc_mat(inputs["qkv_inter_dw_w"], inputs["qkv_inter_dw_b"],
                    NTAP_I)

    ident_f = np.zeros((128, 2, 192), np.float32)
    ident_f[:, 0, 0:128] = np.eye(128)
    ident_f[0:64, 1, 128:192] = np.eye(64)

    mask = np.zeros((192, 192), np.float32)
    for h in range(HEADS):
        mask[h * 24:(h + 1) * 24, h * 24:(h + 1) * 24] = 1.0
    mask_f = np.zeros((128, 2, 192), np.float32)
    mask_f[:, 0, :] = mask[0:128]
    mask_f[0:64, 1, :] = mask[128:192]

    temp_c = np.repeat(temp, C // HEADS)              # [192]
    temp2 = np.zeros((128, 2), np.float32)
    temp2[:, 0] = temp_c[0:128]
    temp2[0:64, 1] = temp_c[128:192]

    bias_out = np.zeros((128, 4), np.float32)
    bi = np.asarray(inputs["proj_inter_b"], np.float32)
    bt = np.asarray(inputs["proj_intra_b"], np.float32)
    bias_out[:, 0] = bi[0:128]
    bias_out[0:64, 1] = bi[128:192]
    bias_out[:, 2] = bt[0:128]
    bias_out[0:64, 3] = bt[128:192]

    def wp_mats(w):
        wt = np.asarray(w, np.float32).reshape(C, C).T         # [c, e]
        return wt[0:128].astype(BF16), wt[128:192].astype(BF16)

    wpT_i_a, wpT_i_b = wp_mats(inputs["proj_inter_w"])
    wpT_t_a, wpT_t_b = wp_mats(inputs["proj_intra_w"])

    shared = {
        "wpw_i_a": wpw_i_a, "wpw_i_b": wpw_i_b,
        "wpw_t_a": wpw_t_a, "wpw_t_b": wpw_t_b,
        "diag_i": diag_i, "diag_t": diag_t, "wsc_i": wsc_i,
        "ident_f": ident_f, "mask_f": mask_f, "temp2": temp2,
        "bias_out": bias_out,
        "wpT_i_a": wpT_i_a, "wpT_i_b": wpT_i_b,
        "wpT_t_a": wpT_t_a, "wpT_t_b": wpT_t_b,
    }

    in_maps = []
    for core in range(N_CORES):
        xs = np.zeros((C, NJ, FR, WP), np.float32)
        msk = np.zeros((NJ, FR, WP), np.float32)
        # inter jobs 0..15: H-slice rows for all frames
        r0 = core * RH
        lo, hi = r0 - 1, r0 + RH + 1
        slo, shi = max(lo, 0), min(hi, H)
        for b in range(B):
            for t in range(T):
                j = b * 8 + t
                xs[:, j, slo - lo:shi - lo, 1:65] = x[b, :, t, slo:shi, :]
                msk[j, slo - lo:shi - lo, 1:65] = 1.0
        # intra jobs 16..31: chunks of own 2 frames
        for Fi in range(2):
            f = core * 2 + Fi
            b, t = f // T, f % T
            for jj in range(8):
                j = 16 + 8 * Fi + jj
                lo2, hi2 = jj * RH - 1, jj * RH + RH + 1
                slo2, shi2 = max(lo2, 0), min(hi2, H)
                xs[:, j, slo2 - lo2:shi2 - lo2, 1:65] = \
                    x[b, :, t, slo2:shi2, :]
                msk[j, slo2 - lo2:shi2 - lo2, 1:65] = 1.0
        xa_c = xs[0:128].reshape(128, NJ, FSZ).astype(BF16)
        xb_c = np.zeros((65, NJ, FSZ), np.float32)
        xb_c[0:64] = xs[128:192].reshape(64, NJ, FSZ)
        xb_c[64] = msk.reshape(NJ, FSZ)
        m = {"xa": xa_c, "xb": xb_c.astype(BF16)}
        m.update(shared)
        in_maps.append(m)
    return in_maps


def _assemble(results):
    out = np.zeros((B, C, T, H, W), np.float32)
    for core in range(N_CORES):
        o_i = np.asarray(results[core]["out_i"],
                         np.float32).reshape(192, 16, RH, W)
        o_t = np.asarray(results[core]["out_t"],
                         np.float32).reshape(192, 16, RH, W)
        r0 = core * RH
        for b in range(B):
            for t in range(T):
                out[b, :, t, r0:r0 + RH, :] += o_i[:, b * 8 + t]
        for Fi in range(2):
            f = core * 2 + Fi
            b, t = f // T, f % T
            for jj in range(8):
                out[b, :, t, jj * RH:jj * RH + RH, :] += o_t[:, 8 * Fi + jj]
    return out


# --------------------------------------------------------------------------
# numpy fallback (known-correct baseline)
# --------------------------------------------------------------------------

def _np_reference(inputs):
    EPS = 1e-12
    x = np.asarray(inputs["x"], np.float32)
    hd = C // HEADS

    def conv_pw3d(xx, w, bb):
        y = np.einsum("oc,bcthw->bothw",
                      np.asarray(w, np.float32)[:, :, 0, 0, 0], xx,
                      optimize=True)
        return y + np.asarray(bb, np.float32)[None, :, None, None, None]

    def conv_dw3d(xx, w, bb):
        xp = np.pad(xx, ((0, 0), (0, 0), (1, 1), (1, 1), (1, 1)))
        out = np.zeros_like(xx)
        w = np.asarray(w, np.float32)
        for dt in range(3):
            for dh in range(3):
                for dw_ in range(3):
                    out += (w[None, :, 0, dt, dh, dw_, None, None, None]
                            * xp[:, :, dt:dt + T, dh:dh + H, dw_:dw_ + W])
        return out + np.asarray(bb, np.float32)[None, :, None, None, None]

    def conv_pw2d(xx, w, bb):
        y = np.einsum("oc,nchw->nohw", np.asarray(w, np.float32)[:, :, 0, 0],
                      xx, optimize=True)
        return y + np.asarray(bb, np.float32)[None, :, None, None]

    def conv_dw2d(xx, w, bb):
        n, cc, hh, ww = xx.shape
        xp = np.pad(xx, ((0, 0), (0, 0), (1, 1), (1, 1)))
        out = np.zeros_like(xx)
        w = np.asarray(w, np.float32)
        for dh in range(3):
            for dw_ in range(3):
                out += (w[None, :, 0, dh, dw_, None, None]
                        * xp[:, :, dh:dh + hh, dw_:dw_ + ww])
        return out + np.asarray(bb, np.float32)[None, :, None, None]

    def l2n(tq):
        n = np.sqrt(np.sum(tq * tq, axis=-1, keepdims=True))
        return tq / np.maximum(n, EPS)

    def chattn(q, k, v, temperature):
        q, k = l2n(q), l2n(k)
        attn = np.einsum("bhcn,bhdn->bhcd", q, k, optimize=True) * temperature
        attn = attn - attn.max(axis=-1, keepdims=True)
        e = np.exp(attn)
        attn = e / e.sum(axis=-1, keepdims=True)
        return np.einsum("bhcd,bhdn->bhcn", attn, v, optimize=True)

    temp = np.asarray(inputs["temperature"], np.float32)
    qkv = conv_dw3d(conv_pw3d(x, inputs["qkv_inter_w"], inputs["qkv_inter_b"]),
                    inputs["qkv_inter_dw_w"], inputs["qkv_inter_dw_b"])
    q, k, v = np.split(qkv, 3, axis=1)
    r3 = lambda tq: tq.reshape(B, HEADS, hd, T * H * W)
    o = chattn(r3(q), r3(k), r3(v), temp)
    out_inter = conv_pw3d(o.reshape(B, C, T, H, W), inputs["proj_inter_w"],
                          inputs["proj_inter_b"])

    x2 = x.transpose(0, 2, 1, 3, 4).reshape(B * T, C, H, W)
    qkv2 = conv_dw2d(conv_pw2d(x2, inputs["qkv_intra_w"],
                               inputs["qkv_intra_b"]),
                     inputs["qkv_intra_dw_w"], inputs["qkv_intra_dw_b"])
    q2, k2, v2 = np.split(qkv2, 3, axis=1)
    r2 = lambda tq: tq.reshape(B * T, HEADS, hd, H * W)
    o2 = chattn(r2(q2), r2(k2), r2(v2), temp)
    out_intra = conv_pw2d(o2.reshape(B * T, C, H, W), inputs["proj_intra_w"],
                          inputs["proj_intra_b"])
    out_intra = out_intra.reshape(B, T, C, H, W).transpose(0, 2, 1, 3, 4)
    return out_inter + out_intra


# --------------------------------------------------------------------------
# entry point
# --------------------------------------------------------------------------

def run_on_device(inputs, trace=False):
    nc = _build_program()
    in_maps = _prep_core_inputs(inputs)
    res = run_bass_kernel_spmd(nc, in_maps, core_ids=list(range(N_CORES)),
                               trace=trace)
    return _assemble(res.results), res


def kernel(**inputs):
    try:
        out, _ = run_on_device(inputs)
        return out
    except Exception:
        import traceback
        traceback.print_exc()
        return _np_reference(inputs)


# revision 36
# speedup vs baseline: 1.1364x; 1.1364x over previous
import os
import sys

for _p in ("/opt/trn_rl_repo", "/root/.axon_site/_ro/trn_rl_repo"):
    if _p not in sys.path:
        sys.path.insert(0, _p)

import numpy as np

import concourse.bass as bass
import concourse.tile as tile
from concourse import bacc, mybir
from concourse.bass_utils import run_bass_kernel_spmd

try:
    import ml_dtypes

    BF16 = ml_dtypes.bfloat16
except Exception:  # pragma: no cover
    BF16 = np.float32

HEADS = 8
N_CORES = 8
B, C, T, H, W = 2, 192, 8, 64, 64
RH = H // N_CORES           # 8 rows per slice/chunk
NJ = 32                     # jobs per core: 16 inter + 16 intra
FR, WP = RH + 2, W + 2      # 10 x 66 padded job tile
FSZ = FR * WP               # 660
OSZ = RH * W                # 512
CC3 = 3 * C                 # 576
CBS = [128, 128, 128, 128, 64]
QK_CBS = (0, 1, 2)
V_CBS = (3, 4)
NTAP_I = 28                 # 27 taps + bias
NTAP_T = 10                 # 9 taps + bias
PAYC = 194

F32 = mybir.dt.float32
BF = mybir.dt.bfloat16

_cached = {}
KS = int(os.environ.get("KS", "5"))
KPAY = int(os.environ.get("KPAY", "1"))   # 0: skip pay TTR evict
KGRAM = int(os.environ.get("KGRAM", "1"))  # 0: skip gram matmuls
KTRN = int(os.environ.get("KTRN", "1"))   # 0: skip dma transposes
# inter-branch DW channel blocks offloaded to DVE (STT chains)
KDVE = tuple(int(c) for c in os.environ.get("KDVE", "").split(",") if c)
KGRP = int(os.environ.get("KGRP", "3"))   # DW job-group size (LDW amortize)


def _taps_inter():
    return [(dt, dh, dw) for dt in (-1, 0, 1) for dh in (-1, 0, 1)
            for dw in (-1, 0, 1)]


def _taps_intra():
    return [(dh, dw) for dh in (-1, 0, 1) for dw in (-1, 0, 1)]


def _build_program():
    key = ("nc", KS)
    if key in _cached:
        return _cached[key]

    nc = bacc.Bacc("TRN2", target_bir_lowering=False, debug=False,
                   num_devices=N_CORES)

    def din(name, shape, dt):
        return nc.dram_tensor(name, shape, dt, kind="ExternalInput").ap()

    xa_d = din("xa", [128, NJ, FSZ], BF)
    xb_d = din("xb", [65, NJ, FSZ], BF)       # row 64 = validity mask
    wpw = {"i": (din("wpw_i_a", [128, CC3], BF), din("wpw_i_b", [65, CC3], BF)),
           "t": (din("wpw_t_a", [128, CC3], BF), din("wpw_t_b", [65, CC3], BF))}
    diag_d = {"i": din("diag_i", [128, 5, NTAP_I, 128], BF),
              "t": din("diag_t", [128, 5, NTAP_T, 128], BF)}
    wsc_d = din("wsc_i", [128, 5, NTAP_I], F32)
    wpT = {"i": (din("wpT_i_a", [128, 192], BF), din("wpT_i_b", [64, 192], BF)),
           "t": (din("wpT_t_a", [128, 192], BF), din("wpT_t_b", [64, 192], BF))}
    temp2_d = din("temp2", [128, 2], F32)
    maskf_d = din("mask_f", [128, 2, 192], F32)
    identf_d = din("ident_f", [128, 2, 192], F32)
    biaso_d = din("bias_out", [128, 4], F32)

    out_i = nc.dram_tensor("out_i", [192, 16, OSZ], F32,
                           kind="ExternalOutput").ap()
    out_t = nc.dram_tensor("out_t", [192, 16, OSZ], F32,
                           kind="ExternalOutput").ap()
    cc_in = nc.dram_tensor("cc_in", [B, 192, PAYC], F32, kind="Internal").ap()
    cc_out = nc.dram_tensor("cc_out", [B, 192, PAYC], F32, kind="Internal",
                            addr_space="Shared").ap()
    rk_scr = nc.dram_tensor("rk_scr", [4, 192], F32, kind="Internal").ap()

    with tile.TileContext(nc) as tc:
        import contextlib

        ctx = contextlib.ExitStack()
        with ctx:
            const = ctx.enter_context(tc.tile_pool(name="const", bufs=1))
            xsp = ctx.enter_context(tc.tile_pool(name="xsp", bufs=2))
            pwp = ctx.enter_context(tc.tile_pool(name="pwp", bufs=6))
            dqp = ctx.enter_context(tc.tile_pool(name="dqp", bufs=KGRP + 1))
            qtp = ctx.enter_context(tc.tile_pool(name="qtp", bufs=KGRP + 1))
            vip = ctx.enter_context(tc.tile_pool(name="vip", bufs=1))
            vtp = ctx.enter_context(tc.tile_pool(name="vtp", bufs=2))
            payp = ctx.enter_context(tc.tile_pool(name="payp", bufs=1))
            smx = ctx.enter_context(tc.tile_pool(name="smx", bufs=1))
            mtp = ctx.enter_context(tc.tile_pool(name="mtp", bufs=2))
            outp = ctx.enter_context(tc.tile_pool(name="outp", bufs=1))
            accp = ctx.enter_context(tc.tile_pool(name="accp", bufs=2))

            ps_pw = ctx.enter_context(
                tc.tile_pool(name="ps_pw", bufs=1 if KGRP >= 3 else 2,
                             space="PSUM"))
            ps_dw = ctx.enter_context(
                tc.tile_pool(name="ps_dw", bufs=max(2, KGRP), space="PSUM"))
            ps_g = ctx.enter_context(
                tc.tile_pool(name="ps_g", bufs=1, space="PSUM"))

            def load(ap_in, dt, tag):
                t_ = const.tile(list(ap_in.shape), dt, tag=tag)
                nc.sync.dma_start(t_[:], ap_in)
                return t_

            wpw_s = {k: (load(v[0], BF, f"wpwa{k}"), load(v[1], BF, f"wpwb{k}"))
                     for k, v in wpw.items()}
            diag_s = {k: load(v, BF, f"diag{k}") for k, v in diag_d.items()}
            wsc_s = load(wsc_d, F32, "wsc")
            wpT_s = {k: (load(v[0], BF, f"wpta{k}"), load(v[1], BF, f"wptb{k}"))
                     for k, v in wpT.items()}
            temp2_s = load(temp2_d, F32, "temp2")
            maskf_s = load(maskf_d, F32, "maskf")
            identf_s = load(identf_d, F32, "identf")
            biaso_s = load(biaso_d, F32, "biaso")

            ones512 = const.tile([128, OSZ], BF, tag="ones512")
            nc.vector.memset(ones512[:], 1.0)
            zf = const.tile([128, FR, WP], BF, tag="zf")
            nc.vector.memset(zf[:], 0.0)

            pay_a = payp.tile([128, 4, PAYC], F32, tag="pay_a")
            pay_b = payp.tile([64, 4, PAYC], F32, tag="pay_b")

            v_i = (vip.tile([128, 16, OSZ], BF, tag="via", name="via"),
                   vip.tile([64, 16, OSZ], BF, tag="vib", name="vib"))

            mt_i = [(mtp.tile([128, 192], BF, tag="mtia", name=f"mtia{b}"),
                     mtp.tile([64, 192], BF, tag="mtib", name=f"mtib{b}"))
                    for b in range(B)]

            evict_ctr = [0]

            def evict(dst_ap, src_ap):
                evict_ctr[0] += 1
                if evict_ctr[0] % 2 == 0:
                    nc.vector.tensor_copy(dst_ap, src_ap)
                else:
                    nc.scalar.copy(dst_ap, src_ap)

            pw_ring = {}

            def do_pw(job):
                br = "i" if job < 16 else "t"
                wa, wb = wpw_s[br]
                xt_a = xsp.tile([128, FSZ], BF, tag="xa")
                xt_b = xsp.tile([65, FSZ], BF, tag="xb")
                nc.sync.dma_start(xt_a[:], xa_d[:, job, :])
                nc.sync.dma_start(xt_b[:], xb_d[:, job, :])
                pwt = pwp.tile([128, 5, FR, WP], BF, tag="pw")
                pw_ring[job] = pwt
                for cb in range(5):
                    psz = CBS[cb]
                    c0 = cb * 128
                    for half in range(2):
                        s0, s1 = half * 330, half * 330 + 330
                        ps = ps_pw.tile([psz, 330], F32, tag="pw")
                        nc.tensor.matmul(ps[:], wa[:, c0:c0 + psz],
                                         xt_a[:, s0:s1], start=True, stop=False)
                        nc.tensor.matmul(ps[:], wb[:, c0:c0 + psz],
                                         xt_b[:, s0:s1], start=False, stop=True)
                        dst = pwt[:psz, cb].rearrange("p a b -> p (a b)")
                        evict(dst[:, s0:s1], ps[:])

            def dw_moving(job, cb, tap):
                # shifted view of pw output for one tap (or None -> zeros)
                psz = CBS[cb]
                br_inter = job < 16
                if br_inter:
                    dt_, dh, dw = tap
                    b, t = job // 8, job % 8
                    tt = t + dt_
                    if 0 <= tt < T:
                        src = pw_ring[b * 8 + tt]
                        return src[:psz, cb, 1 + dh:9 + dh, 1 + dw:65 + dw]
                    return zf[:psz, 1 + dh:9 + dh, 1 + dw:65 + dw]
                dh, dw = tap
                return pw_ring[job][:psz, cb, 1 + dh:9 + dh, 1 + dw:65 + dw]

            def _dw_dve(job, cb, taps, ntap):
                # STT chain on DVE: dq = sum_t w_t * shift_t(pw) + bias
                psz = CBS[cb]
                acc = [accp.tile([128, OSZ], BF, tag="acc0", name="acc0"),
                       accp.tile([128, OSZ], BF, tag="acc1", name="acc1")]
                dq = dqp.tile([128, OSZ], BF, tag="dq")
                nc.vector.tensor_scalar(
                    acc[0][:psz], dw_moving(job, cb, taps[0]),
                    wsc_s[:psz, cb, 0:1], wsc_s[:psz, cb, ntap - 1:ntap],
                    op0=mybir.AluOpType.mult, op1=mybir.AluOpType.add)
                cur = 0
                for ti in range(1, len(taps)):
                    last = ti == len(taps) - 1
                    out_ap = dq[:psz] if last else acc[1 - cur][:psz]
                    nc.vector.scalar_tensor_tensor(
                        out_ap, dw_moving(job, cb, taps[ti]),
                        wsc_s[:psz, cb, ti:ti + 1], acc[cur][:psz],
                        op0=mybir.AluOpType.mult, op1=mybir.AluOpType.add)
                    cur = 1 - cur
                return dq

            def _qk_store(job, cb, dq, qkTs):
                c0 = cb * 128
                psz = CBS[cb]
                if KS < 3:
                    return
                if job not in qkTs:
                    qkTs[job] = qtp.tile([128, 4, 384], BF, tag="qkT",
                                         name=f"qkT{job}")
                    if KTRN == 0:
                        nc.vector.memset(qkTs[job][:], 0.0)
                if KTRN:
                    for ch in range(4):
                        nc.sync.dma_start_transpose(
                            qkTs[job][:, ch, c0:c0 + psz],
                            dq[:psz, ch * 128:(ch + 1) * 128])

            def do_dw_gram(jobs, v_cur):
                # depthwise + gram for a group of jobs sharing diag weights
                br = "i" if jobs[0] < 16 else "t"
                taps = _taps_inter() if br == "i" else _taps_intra()
                ntap = len(taps) + 1
                dg = diag_s[br]
                qkTs = {}
                for cb in range(5):
                    psz = CBS[cb]
                    use_dve = (br == "i" and cb in KDVE and cb in QK_CBS
                               and KS >= 2)
                    if use_dve:
                        for job in jobs:
                            dq = _dw_dve(job, cb, taps, ntap)
                            _qk_store(job, cb, dq, qkTs)
                        continue
                    dps = {j: ps_dw.tile([psz, OSZ], F32, tag="dw",
                                         name=f"dw{j % KGRP}")
                           for j in jobs}
                    for ti in range(ntap):
                        for j in jobs:
                            if ti < len(taps):
                                r_ap = dw_moving(j, cb, taps[ti])
                            else:
                                r_ap = ones512[:psz, :]
                            nc.tensor.matmul(dps[j][:], dg[:psz, cb, ti, :psz],
                                             r_ap, start=(ti == 0),
                                             stop=(ti == ntap - 1))
                    if KS < 2:
                        continue
                    for job in jobs:
                        if cb in QK_CBS:
                            dq = dqp.tile([128, OSZ], BF, tag="dq")
                            evict(dq[:psz], dps[job][:])
                            _qk_store(job, cb, dq, qkTs)
                        else:
                            va, vb = v_cur
                            slot = job if job < 16 else (job - 16) % 8
                            dst = va if cb == 3 else vb
                            evict(dst[:psz, slot, :], dps[job][:])
                if KS < 3 or KGRAM == 0:
                    return
                for job in jobs:
                    inst = job // 8
                    g_qa, g_qb, g_ka, g_kb = gram_tiles[inst]
                    qkT = qkTs[job]
                    for ch in range(4):
                        qt = qkT[:, ch, :]
                        st = job % 8 == 0 and ch == 0
                        sp = job % 8 == 7 and ch == 3
                        # [q.q | q.k] fused: cols 0:192 = qq, 192:384 = qk
                        nc.tensor.matmul(g_qa[:], qt[:, 0:128],
                                         qt[:, 0:384], start=st, stop=sp)
                        nc.tensor.matmul(g_qb[:], qt[:, 128:192],
                                         qt[:, 0:384], start=st, stop=sp)
                        nc.tensor.matmul(g_ka[:], qt[:, 192:320],
                                         qt[:, 192:384], start=st, stop=sp)
                        nc.tensor.matmul(g_kb[:], qt[:, 320:384],
                                         qt[:, 192:384], start=st, stop=sp)

            gram_tiles = {}

            def alloc_gram(inst):
                gram_tiles[inst] = (
                    ps_g.tile([128, 384], F32, tag="gqa", name=f"gqa{inst}"),
                    ps_g.tile([64, 384], F32, tag="gqb", name=f"gqb{inst}"),
                    ps_g.tile([128, 192], F32, tag="gka", name=f"gka{inst}"),
                    ps_g.tile([64, 192], F32, tag="gkb", name=f"gkb{inst}"),
                )

            def pay_evict(inst):
                if KPAY == 0 or KGRAM == 0:
                    return
                g_qa, g_qb, g_ka, g_kb = gram_tiles[inst]
                nc.vector.tensor_copy(pay_a[:, inst, 0:192], g_qa[:, 192:384])
                nc.scalar.copy(pay_b[:, inst, 0:192], g_qb[:, 192:384])
                tmp = smx.tile([128, 192], F32, tag="diag_tmp")
                for (g, ident, dst) in (
                        (g_qa[:, 0:192], identf_s[:, 0, :],
                         pay_a[:, inst, 192:193]),
                        (g_qb[:, 0:192], identf_s[0:64, 1, :],
                         pay_b[:, inst, 192:193]),
                        (g_ka[:], identf_s[:, 0, :],
                         pay_a[:, inst, 193:194]),
                        (g_kb[:], identf_s[0:64, 1, :],
                         pay_b[:, inst, 193:194])):
                    p = g.shape[0]
                    nc.vector.tensor_mul(tmp[:p], g, ident)
                    nc.vector.tensor_reduce(dst, tmp[:p],
                                            axis=mybir.AxisListType.X,
                                            op=mybir.AluOpType.add)

            def posts(inst, br, pa, pb, mt_a, mt_b):
                # pa [128, PAYC] f32 AP; pb [64, PAYC]
                rq_a = smx.tile([128, 1], F32, tag="rq_a")
                rq_b = smx.tile([64, 1], F32, tag="rq_b")
                rk_a = smx.tile([128, 1], F32, tag="rk_a")
                rk_b = smx.tile([64, 1], F32, tag="rk_b")
                for (dst, src, pt, p) in ((rq_a, pa, 0, 128), (rq_b, pb, 1, 64)):
                    nc.scalar.sqrt(dst[:], src[:, 192:193])
                    nc.vector.reciprocal(dst[:], dst[:])
                    nc.vector.tensor_scalar_mul(dst[:], dst[:],
                                                temp2_s[:p, pt:pt + 1])
                for (dst, src) in ((rk_a, pa), (rk_b, pb)):
                    nc.scalar.sqrt(dst[:], src[:, 193:194])
                    nc.vector.reciprocal(dst[:], dst[:])
                nc.sync.dma_start(rk_scr[inst, 0:128], rk_a[:])
                nc.sync.dma_start(rk_scr[inst, 128:192], rk_b[:])
                rkb_a = smx.tile([128, 192], F32, tag="rkb_a")
                rkb_b = smx.tile([64, 192], F32, tag="rkb_b")
                nc.sync.dma_start(
                    rkb_a[:], bass.AP(tensor=rk_scr.tensor,
                                      offset=rk_scr.offset + inst * 192,
                                      ap=[[0, 128], [1, 192]]))
                nc.sync.dma_start(
                    rkb_b[:], bass.AP(tensor=rk_scr.tensor,
                                      offset=rk_scr.offset + inst * 192,
                                      ap=[[0, 64], [1, 192]]))
                attn_a = smx.tile([128, 192], BF, tag="attn_a")
                attn_b = smx.tile([64, 192], BF, tag="attn_b")
                for (pt, p, post, rqv, rkb, attn) in (
                        (0, 128, pa, rq_a, rkb_a, attn_a),
                        (1, 64, pb, rq_b, rkb_b, attn_b)):
                    s = smx.tile([128, 192], F32, tag="s_f32")
                    nc.vector.tensor_scalar_mul(s[:p], post[:, 0:192], rqv[:p])
                    nc.vector.tensor_mul(s[:p], s[:p], rkb[:p])
                    nc.scalar.activation(s[:p], s[:p],
                                         mybir.ActivationFunctionType.Exp)
                    rs = smx.tile([128, 1], F32, tag="rs")
                    e = smx.tile([128, 192], F32, tag="e_f32")
                    nc.vector.tensor_mul(e[:p], s[:p], maskf_s[:p, pt, :])
                    nc.vector.tensor_reduce(rs[:p], e[:p],
                                            axis=mybir.AxisListType.X,
                                            op=mybir.AluOpType.add)
                    nc.vector.reciprocal(rs[:p], rs[:p])
                    nc.vector.tensor_scalar_mul(attn[:], e[:p], rs[:p])
                wpa, wpb = wpT_s[br]
                mps_a = ps_g.tile([128, 192], F32, tag="gka")
                mps_b = ps_g.tile([64, 192], F32, tag="gkb")
                nc.tensor.matmul(mps_a[:], attn_a[:, 0:128], wpa[:],
                                 start=True, stop=False)
                nc.tensor.matmul(mps_a[:], attn_b[:, 0:128], wpb[:],
                                 start=False, stop=True)
                nc.tensor.matmul(mps_b[:], attn_a[:, 128:192], wpa[:],
                                 start=True, stop=False)
                nc.tensor.matmul(mps_b[:], attn_b[:, 128:192], wpb[:],
                                 start=False, stop=True)
                evict(mt_a[:], mps_a[:])
                evict(mt_b[:], mps_b[:])

            def attnv(job_slot, mt_a, mt_b, va, vb, out_d, out_slot, bc):
                eo_a = ps_dw.tile([128, OSZ], F32, tag="dw")
                eo_b = ps_dw.tile([64, OSZ], F32, tag="dw")
                nc.tensor.matmul(eo_a[:], mt_a[:, 0:128], va[:, job_slot, :],
                                 start=True, stop=False)
                nc.tensor.matmul(eo_a[:], mt_b[:, 0:128], vb[:, job_slot, :],
                                 start=False, stop=True)
                nc.tensor.matmul(eo_b[:], mt_a[:, 128:192], va[:, job_slot, :],
                                 start=True, stop=False)
                nc.tensor.matmul(eo_b[:], mt_b[:, 128:192], vb[:, job_slot, :],
                                 start=False, stop=True)
                oa = outp.tile([128, OSZ], F32, tag="oa")
                ob = outp.tile([64, OSZ], F32, tag="ob")
                nc.scalar.activation(oa[:], eo_a[:],
                                     mybir.ActivationFunctionType.Identity,
                                     bias=biaso_s[:, bc:bc + 1], scale=1.0)
                nc.scalar.activation(ob[:], eo_b[:],
                                     mybir.ActivationFunctionType.Identity,
                                     bias=biaso_s[0:64, bc + 1:bc + 2],
                                     scale=1.0)
                nc.sync.dma_start(out_d[0:128, out_slot, :], oa[:])
                nc.sync.dma_start(out_d[128:192, out_slot, :], ob[:])

            # ================= phase A: inter =========================
            groups = [tuple(range(i, min(i + KGRP, 8)))
                      for i in range(0, 8, KGRP)]
            for b in range(B):
                alloc_gram(b)
                base = b * 8
                issued = 0
                for g in groups:
                    hi = min(7, g[-1] + 1)
                    while issued <= hi:
                        do_pw(base + issued)
                        issued += 1
                    do_dw_gram(tuple(base + t for t in g), v_i)
                if KS >= 3:
                    pay_evict(b)
            if KS >= 4:
                nc.sync.dma_start(
                    cc_in[:, 0:128, :].rearrange("i p c -> p i c"),
                    pay_a[:, 0:B, :])
                nc.sync.dma_start(
                    cc_in[:, 128:192, :].rearrange("i p c -> p i c"),
                    pay_b[:, 0:B, :])
                nc.gpsimd.collective_compute(
                    "AllReduce", mybir.AluOpType.add,
                    replica_groups=[list(range(N_CORES))],
                    ins=[cc_in[:]], outs=[cc_out[:]])

            def phase_c():
                # inter posts + attn@v; issued mid-phase-B so it overlaps
                for b in range(B):
                    pa = smx.tile([128, PAYC], F32, tag="cc_a")
                    pb = smx.tile([64, PAYC], F32, tag="cc_b")
                    nc.sync.dma_start(pa[:], cc_out[b, 0:128, :])
                    nc.sync.dma_start(pb[:], cc_out[b, 128:192, :])
                    posts(b, "i", pa[:], pb[:], mt_i[b][0], mt_i[b][1])
                for job in range(16):
                    b = job // 8
                    attnv(job, mt_i[b][0], mt_i[b][1], v_i[0], v_i[1],
                          out_i, job, 0)

            # ================= phase B: intra =========================
            for F in range(2):
                inst = 2 + F
                alloc_gram(inst)
                v_cur = (vtp.tile([128, 8, OSZ], BF, tag="vta",
                                  name=f"vta{F}"),
                         vtp.tile([64, 8, OSZ], BF, tag="vtb",
                                  name=f"vtb{F}"))
                for g in groups:
                    gjobs = tuple(16 + 8 * F + jj for jj in g)
                    for job in gjobs:
                        do_pw(job)
                    do_dw_gram(gjobs, v_cur)
                if KS >= 3:
                    pay_evict(inst)
                if KS >= 5:
                    mt_t_a = mtp.tile([128, 192], BF, tag="mtta")
                    mt_t_b = mtp.tile([64, 192], BF, tag="mttb")
                    posts(inst, "t", pay_a[:, inst, :], pay_b[:, inst, :],
                          mt_t_a, mt_t_b)
                    for jj in range(8):
                        attnv(jj, mt_t_a, mt_t_b, v_cur[0], v_cur[1],
                              out_t, 8 * F + jj, 2)

            # ================= phase C: inter posts ===================
            if KS >= 5:
                phase_c()

    nc.compile()
    _cached[key] = nc
    return nc


# --------------------------------------------------------------------------
# host-side input prep
# --------------------------------------------------------------------------

def _prep_core_inputs(inputs):
    x = np.asarray(inputs["x"], np.float32)           # [B, C, T, H, W]
    temp = np.asarray(inputs["temperature"], np.float32).reshape(HEADS)

    def pw_mats(w, bvec):
        wt = np.asarray(w, np.float32).reshape(CC3, C).T       # [ic, oc]
        wa = wt[0:128]
        wb = np.zeros((65, CC3), np.float32)
        wb[0:64] = wt[128:192]
        wb[64] = np.asarray(bvec, np.float32)
        return wa.astype(BF16), wb.astype(BF16)

    wpw_i_a, wpw_i_b = pw_mats(inputs["qkv_inter_w"], inputs["qkv_inter_b"])
    wpw_t_a, wpw_t_b = pw_mats(inputs["qkv_intra_w"], inputs["qkv_intra_b"])

    def diag_mat(w, bvec, ntap):
        w = np.asarray(w, np.float32).reshape(CC3, -1)         # [576, taps-1]
        bvec = np.asarray(bvec, np.float32)
        arr = np.zeros((128, 5, ntap, 128), np.float32)
        for cb in range(5):
            psz = CBS[cb]
            for p in range(psz):
                ch = cb * 128 + p
                arr[p, cb, :ntap - 1, p] = w[ch]
                arr[p, cb, ntap - 1, p] = bvec[ch]
        return arr.astype(BF16)

    diag_i = diag_mat(inputs["qkv_inter_dw_w"], inputs["qkv_inter_dw_b"],
                      NTAP_I)
    diag_t = diag_mat(inputs["qkv_intra_dw_w"], inputs["qkv_intra_dw_b"],
                      NTAP_T)

    def wsc_mat(w, bvec, ntap):
        w = np.asarray(w, np.float32).reshape(CC3, -1)
        bvec = np.asarray(bvec, np.float32)
        arr = np.zeros((128, 5, ntap), np.float32)
        for cb in range(5):
            psz = CBS[cb]
            arr[:psz, cb, :ntap - 1] = w[cb * 128:cb * 128 + psz]
            arr[:psz, cb, ntap - 1] = bvec[cb * 128:cb * 128 + psz]
        return arr

    wsc_i = wsc_mat(inputs["qkv_inter_dw_w"], inputs["qkv_inter_dw_b"],
                    NTAP_I)

    ident_f = np.zeros((128, 2, 192), np.float32)
    ident_f[:, 0, 0:128] = np.eye(128)
    ident_f[0:64, 1, 128:192] = np.eye(64)

    mask = np.zeros((192, 192), np.float32)
    for h in range(HEADS):
        mask[h * 24:(h + 1) * 24, h * 24:(h + 1) * 24] = 1.0
    mask_f = np.zeros((128, 2, 192), np.float32)
    mask_f[:, 0, :] = mask[0:128]
    mask_f[0:64, 1, :] = mask[128:192]

    temp_c = np.repeat(temp, C // HEADS)              # [192]
    temp2 = np.zeros((128, 2), np.float32)
    temp2[:, 0] = temp_c[0:128]
    temp2[0:64, 1] = temp_c[128:192]

    bias_out = np.zeros((128, 4), np.float32)
    bi = np.asarray(inputs["proj_inter_b"], np.float32)
    bt = np.asarray(inputs["proj_intra_b"], np.float32)
    bias_out[:, 0] = bi[0:128]
    bias_out[0:64, 1] = bi[128:192]
    bias_out[:, 2] = bt[0:128]
    bias_out[0:64, 3] = bt[128:192]

    def wp_mats(w):
        wt = np.asarray(w, np.float32).reshape(C, C).T         # [c, e]
        return wt[0:128].astype(BF16), wt[128:192].astype(BF16)

    wpT_i_a, wpT_i_b = wp_mats(inputs["proj_inter_w"])
    wpT_t_a, wpT_t_b = wp_mats(inputs["proj_intra_w"])

    shared = {
        "wpw_i_a": wpw_i_a, "wpw_i_b": wpw_i_b,
        "wpw_t_a": wpw_t_a, "wpw_t_b": wpw_t_b,
        "diag_i": diag_i, "diag_t": diag_t, "wsc_i": wsc_i,
        "ident_f": ident_f, "mask_f": mask_f, "temp2": temp2,
        "bias_out": bias_out,
        "wpT_i_a": wpT_i_a, "wpT_i_b": wpT_i_b,
        "wpT_t_a": wpT_t_a, "wpT_t_b": wpT_t_b,
    }

    in_maps = []
    for core in range(N_CORES):
        xs = np.zeros((C, NJ, FR, WP), np.float32)
        msk = np.zeros((NJ, FR, WP), np.float32)
        # inter jobs 0..15: H-slice rows for all frames
        r0 = core * RH
        lo, hi = r0 - 1, r0 + RH + 1
        slo, shi = max(lo, 0), min(hi, H)
        for b in range(B):
            for t in range(T):
                j = b * 8 + t
                xs[:, j, slo - lo:shi - lo, 1:65] = x[b, :, t, slo:shi, :]
                msk[j, slo - lo:shi - lo, 1:65] = 1.0
        # intra jobs 16..31: chunks of own 2 frames
        for Fi in range(2):
            f = core * 2 + Fi
            b, t = f // T, f % T
            for jj in range(8):
                j = 16 + 8 * Fi + jj
                lo2, hi2 = jj * RH - 1, jj * RH + RH + 1
                slo2, shi2 = max(lo2, 0), min(hi2, H)
                xs[:, j, slo2 - lo2:shi2 - lo2, 1:65] = \
                    x[b, :, t, slo2:shi2, :]
                msk[j, slo2 - lo2:shi2 - lo2, 1:65] = 1.0
        xa_c = xs[0:128].reshape(128, NJ, FSZ).astype(BF16)
        xb_c = np.zeros((65, NJ, FSZ), np.float32)
        xb_c[0:64] = xs[128:192].reshape(64, NJ, FSZ)
        xb_c[64] = msk.reshape(NJ, FSZ)
        m = {"xa": xa_c, "xb": xb_c.astype(BF16)}
        m.update(shared)
        in_maps.append(m)
    return in_maps


def _assemble(results):
    out = np.zeros((B, C, T, H, W), np.float32)
    for core in range(N_CORES):
        o_i = np.asarray(results[core]["out_i"],
                         np.float32).reshape(192, 16, RH, W)
        o_t = np.asarray(results[core]["out_t"],
                         np.float32).reshape(192, 16, RH, W)
        r0 = core * RH
        for b in range(B):
            for t in range(T):
                out[b, :, t, r0:r0 + RH, :] += o_i[:, b * 8 + t]
        for Fi in range(2):
            f = core * 2 + Fi
            b, t = f // T, f % T
            for jj in range(8):
                out[b, :, t, jj * RH:jj * RH + RH, :] += o_t[:, 8 * Fi + jj]
    return out


# --------------------------------------------------------------------------
# numpy fallback (known-correct baseline)
# --------------------------------------------------------------------------

def _np_reference(inputs):
    EPS = 1e-12
    x = np.asarray(inputs["x"], np.float32)
    hd = C // HEADS

    def conv_pw3d(xx, w, bb):
        y = np.einsum("oc,bcthw->bothw",
                      np.asarray(w, np.float32)[:, :, 0, 0, 0], xx,
                      optimize=True)
        return y + np.asarray(bb, np.float32)[None, :, None, None, None]

    def conv_dw3d(xx, w, bb):
        xp = np.pad(xx, ((0, 0), (0, 0), (1, 1), (1, 1), (1, 1)))
        out = np.zeros_like(xx)
        w = np.asarray(w, np.float32)
        for dt in range(3):
            for dh in range(3):
                for dw_ in range(3):
                    out += (w[None, :, 0, dt, dh, dw_, None, None, None]
                            * xp[:, :, dt:dt + T, dh:dh + H, dw_:dw_ + W])
        return out + np.asarray(bb, np.float32)[None, :, None, None, None]

    def conv_pw2d(xx, w, bb):
        y = np.einsum("oc,nchw->nohw", np.asarray(w, np.float32)[:, :, 0, 0],
                      xx, optimize=True)
        return y + np.asarray(bb, np.float32)[None, :, None, None]

    def conv_dw2d(xx, w, bb):
        n, cc, hh, ww = xx.shape
        xp = np.pad(xx, ((0, 0), (0, 0), (1, 1), (1, 1)))
        out = np.zeros_like(xx)
        w = np.asarray(w, np.float32)
        for dh in range(3):
            for dw_ in range(3):
                out += (w[None, :, 0, dh, dw_, None, None]
                        * xp[:, :, dh:dh + hh, dw_:dw_ + ww])
        return out + np.asarray(bb, np.float32)[None, :, None, None]

    def l2n(tq):
        n = np.sqrt(np.sum(tq * tq, axis=-1, keepdims=True))
        return tq / np.maximum(n, EPS)

    def chattn(q, k, v, temperature):
        q, k = l2n(q), l2n(k)
        attn = np.einsum("bhcn,bhdn->bhcd", q, k, optimize=True) * temperature
        attn = attn - attn.max(axis=-1, keepdims=True)
        e = np.exp(attn)
        attn = e / e.sum(axis=-1, keepdims=True)
        return np.einsum("bhcd,bhdn->bhcn", attn, v, optimize=True)

    temp = np.asarray(inputs["temperature"], np.float32)
    qkv = conv_dw3d(conv_pw3d(x, inputs["qkv_inter_w"], inputs["qkv_inter_b"]),
                    inputs["qkv_inter_dw_w"], inputs["qkv_inter_dw_b"])
    q, k, v = np.split(qkv, 3, axis=1)
    r3 = lambda tq: tq.reshape(B, HEADS, hd, T * H * W)
    o = chattn(r3(q), r3(k), r3(v), temp)
    out_inter = conv_pw3d(o.reshape(B, C, T, H, W), inputs["proj_inter_w"],
                          inputs["proj_inter_b"])

    x2 = x.transpose(0, 2, 1, 3, 4).reshape(B * T, C, H, W)
    qkv2 = conv_dw2d(conv_pw2d(x2, inputs["qkv_intra_w"],
                               inputs["qkv_intra_b"]),
                     inputs["qkv_intra_dw_w"], inputs["qkv_intra_dw_b"])
    q2, k2, v2 = np.split(qkv2, 3, axis=1)
    r2 = lambda tq: tq.reshape(B * T, HEADS, hd, H * W)
    o2 = chattn(r2(q2), r2(k2), r2(v2), temp)
    out_intra = conv_pw2d(o2.reshape(B * T, C, H, W), inputs["proj_intra_w"],
                          inputs["proj_intra_b"])
    out_intra = out_intra.reshape(B, T, C, H, W).transpose(0, 2, 1, 3, 4)
    return out_inter + out_intra


# --------------------------------------------------------------------------
# entry point
# --------------------------------------------------------------------------

def run_on_device(inputs, trace=False):
    nc = _build_program()
    in_maps = _prep_core_inputs(inputs)
    res = run_bass_kernel_spmd(nc, in_maps, core_ids=list(range(N_CORES)),
                               trace=trace)
    return _assemble(res.results), res


def kernel(**inputs):
    try:
        out, _ = run_on_device(inputs)
        return out
    except Exception:
        import traceback
        traceback.print_exc()
        return _np_reference(inputs)
